# revision 1
# baseline (speedup 1.0000x reference)
"""Trainium2 Bass kernel for nn_MemoryEfficientNonLinearConv2d.

Math: per conv term, current = ALPHA*(msp(t1)^2 - msp(t2)^2) with
t1=(V-w)/c, t2=t1-4/3, msp(t)=log1p(exp(clip(t,-20,20))) masked at -20.
V=clip(x,0,10), x~U[0,1): each term is a 1-D function h(V-w) of V.

Decomposition h = htilde - ALPHA*kappa:
 - htilde (unclipped softplus form) is analytic; fit per-weight in a shared
   34-row sigmoid basis of V (max fit err ~4e-8). Conv becomes 9 shifted
   float32r matmuls contracting (basis x cin) against per-weight coeffs.
 - kappa (the clip at t=20; exact in fp32 because softplus(t)=t for t>=16):
   kappa = m*(2*u1 - m + 40), u1=relu((V-w-1.5)/c), m=min(u1, 4/3).
   Only weights with w < xmax-1.5 ("risky", ~25%) have kappa != 0; each
   gets ONE matmul row (the product), built with 1 ACT relu + cheap
   DVE/GPSIMD tensor ops, coefficient -RG*ALPHA at (its co, its shift).

Sharding: output pixels by oh-bands of 4 rows across 8 cores (M=64
channels per matmul, N=512 pixels = one PSUM bank). BatchNorm uses
per-core partial sums + a [64,2] AllReduce, then normalize+clip fused
into a per-partition Relu + min. Output gathered on host.

All weight-dependent structure (fit coefficients, risky packing, biases,
lhsT matrices) is computed on host with numpy at call time.
"""
import sys
import os
import numpy as np

for _p in ("/opt/trn_rl_repo", "/root/.axon_site/_ro/trn_rl_repo"):
    if os.path.isdir(_p) and _p not in sys.path:
        sys.path.insert(0, _p)

import concourse.bass as bass
import concourse.bacc as bacc
import concourse.mybir as mybir
import concourse.tile as tile
from concourse.bass_utils import run_bass_kernel_spmd
from contextlib import ExitStack

AF = mybir.ActivationFunctionType
ALU = mybir.AluOpType
DT = mybir.dt

ALPHA = 0.0005625
C = 0.075
VD = 0.1
RG = 0.1
DELTA = VD / C  # 4/3
BN_EPS = 1e-5
B, CIN, H, W = 4, 32, 32, 32
COUT = 64
OH = OW = 32
NCORES = 8
KB = 32           # sigmoid grid knots
SIG_S = 30.0
MARGIN = 0.2
NBASIS = KB + 2   # + const-sigmoid + wide-sigmoid
NGRID_TILES = (NBASIS + 3) // 4  # 9 tiles of (slot=4, ci=32) rows
SLAB_FREE = B * 6 * 34           # 816
NPIX = B * 4 * OW                # 512 output pixels per core


def _sp64(t):
    return np.where(t > 30, t, np.log1p(np.exp(np.minimum(t, 30.0))))


def _htilde64(d):
    return ALPHA * (_sp64(d / C) ** 2 - _sp64((d - VD) / C) ** 2)


def _host_prep(x, theta):
    x = np.asarray(x, np.float32)
    theta = np.asarray(theta, np.float32)
    xc = np.clip(x, 0.0, 10.0)
    xmax = float(xc.max())
    vhi = max(1.0, xmax * 1.0000001)

    # sigmoid basis rows: sigmoid(scale*V + bias)
    knots = np.linspace(-MARGIN, vhi + MARGIN, KB)
    scales = np.r_[0.0, 1.5, np.full(KB, SIG_S)]
    biases = np.r_[25.0, -1.5 * vhi / 2.0, -SIG_S * knots]

    # fp64 fit of RG*htilde(V - w) for every weight
    Vfit = np.linspace(0.0, vhi, 1501)
    A = 1.0 / (1.0 + np.exp(-(Vfit[:, None] * scales[None, :] + biases[None, :])))
    wflat = theta.astype(np.float64).ravel()
    G = RG * _htilde64(Vfit[:, None] - wflat[None, :])
    lam = 1e-12 * np.trace(A.T @ A) / A.shape[1]
    coef = np.linalg.solve(A.T @ A + lam * np.eye(A.shape[1]), A.T @ G)
    active = (wflat > -1.6) & (wflat < 2.5)
    coef = coef * active[None, :]

    th4 = theta.reshape(COUT, CIN, 3, 3).astype(np.float64)

    # risky weights, shift-clustered greedy packing per ci
    percil = [[] for _ in range(CIN)]
    for ci in range(CIN):
        lst = []
        for kh in range(3):
            for kw in range(3):
                for co in range(COUT):
                    w = th4[co, ci, kh, kw]
                    if (w > -1.6) and (w + 1.5 < xmax):
                        lst.append((kh * 3 + kw, co, w))
        lst.sort()
        percil[ci] = lst
    nm_tiles = max(1, max((len(v) + 3) // 4 for v in percil))
    mbias = np.full((nm_tiles, 128), -1e9, np.float32)  # ACT relu bias; empty -> 0
    slot_map = [dict() for _ in range(nm_tiles)]        # tile -> {part: (shift, co)}
    unions = [set() for _ in range(nm_tiles)]
    for ci in range(CIN):
        for j, (shift, co, w) in enumerate(percil[ci]):
            t, s = divmod(j, 4)
            p = s * 32 + ci
            mbias[t, p] = -(w + 1.5) / C
            slot_map[t][p] = (shift, co)
            unions[t].add(shift)

    # matmul pair list: (tile_idx, shift); grid tiles take all 9 shifts
    pairs = []
    for t in range(NGRID_TILES):
        for sh in range(9):
            pairs.append((t, sh))
    for t in range(nm_tiles):
        for sh in sorted(unions[t]):
            pairs.append((NGRID_TILES + t, sh))

    # lhsT per pair
    NP = len(pairs)
    lhsT = np.zeros((NP, 128, COUT), np.float32)
    wi_all = {}
    for kh in range(3):
        for kw in range(3):
            wi_all[kh * 3 + kw] = (
                (np.arange(COUT)[:, None] * CIN + np.arange(CIN)[None, :]) * 3
                + kh) * 3 + kw
    for pi, (t, sh) in enumerate(pairs):
        if t < NGRID_TILES:
            for slot in range(4):
                k = 4 * t + slot
                if k >= NBASIS:
                    continue
                lhsT[pi, slot * 32:(slot + 1) * 32, :] = \
                    coef[k, wi_all[sh]].T.astype(np.float32)
        else:
            for p, (shift, co) in slot_map[t - NGRID_TILES].items():
                if shift == sh:
                    lhsT[pi, p, co] = -RG * ALPHA

    # consts [nconst, 128]: sigma (scale,bias) x NGRID_TILES, then m-tile biases
    nconst = 2 * NGRID_TILES + nm_tiles
    consts = np.zeros((nconst, 128), np.float32)
    for t in range(NGRID_TILES):
        for slot in range(4):
            k = 4 * t + slot
            sc, bi = (scales[k], biases[k]) if k < NBASIS else (0.0, 25.0)
            consts[2 * t, slot * 32:(slot + 1) * 32] = sc
            consts[2 * t + 1, slot * 32:(slot + 1) * 32] = bi
    consts[2 * NGRID_TILES:] = mbias

    # per-core padded slabs
    x_pad = np.zeros((B, CIN, H + 2, W + 2), np.float32)
    x_pad[:, :, 1:-1, 1:-1] = xc
    slabs = [np.ascontiguousarray(x_pad[:, :, 4 * s:4 * s + 6, :])
             for s in range(NCORES)]

    return dict(slabs=slabs, lhsT=lhsT, consts=consts,
                nm_tiles=nm_tiles, pairs=tuple(pairs), nconst=nconst)


def _build_program(nm_tiles, pairs, nconst, reps=1, no_cc=False):
    NP = len(pairs)
    nc = bacc.Bacc("TRN2", target_bir_lowering=False, debug=False,
                   num_devices=NCORES)

    xslab = nc.dram_tensor("xslab", [B, CIN, 6, 34], DT.float32,
                           kind="ExternalInput").ap()
    lhsT_d = nc.dram_tensor("lhsT", [NP, 128, COUT], DT.float32r,
                            kind="ExternalInput").ap()
    consts_d = nc.dram_tensor("consts", [nconst, 128], DT.float32,
                              kind="ExternalInput").ap()
    gb_d = nc.dram_tensor("gb", [4, COUT], DT.float32,
                          kind="ExternalInput").ap()
    out_d = nc.dram_tensor("out", [reps, COUT, NPIX], DT.float32,
                           kind="ExternalOutput").ap()

    with tile.TileContext(nc) as tc, ExitStack() as ctx:
        cpool = ctx.enter_context(tc.tile_pool(name="cpool", bufs=1))
        upool = ctx.enter_context(tc.tile_pool(name="upool", bufs=1))
        mrpool = ctx.enter_context(tc.tile_pool(name="mrpool", bufs=6))
        spool = ctx.enter_context(tc.tile_pool(name="spool", bufs=2))
        bpool = ctx.enter_context(tc.tile_pool(name="bpool", bufs=2))
        psum = ctx.enter_context(tc.tile_pool(name="psum", bufs=2, space="PSUM"))
        dram = ctx.enter_context(tc.tile_pool(name="dram", bufs=2, space="DRAM"))

        consts_t = cpool.tile([128, nconst], DT.float32)
        nc.sync.dma_start(consts_t[:], consts_d.transpose([1, 0]))
        gb_t = cpool.tile([COUT, 4], DT.float32)
        nc.sync.dma_start(gb_t[:], gb_d.transpose([1, 0]))
        lhsT_t = cpool.tile([128, NP * COUT], DT.float32r)
        nc.sync.dma_start(
            lhsT_t[:].rearrange("p (t m) -> p t m", t=NP),
            lhsT_d.transpose([1, 0, 2]))
        x_rep = cpool.tile([128, SLAB_FREE], DT.float32)
        for slot in range(4):
            nc.sync.dma_start(
                x_rep[slot * 32:(slot + 1) * 32].rearrange(
                    "p (b h w) -> p b h w", b=B, h=6),
                xslab.transpose([1, 0, 2, 3]))

        def build_rep(rep):
            """Emit basis build + matmuls + stats + collective for one rep.
            Returns state needed by bn_tail."""
            acc = psum.tile([COUT, NPIX], DT.float32, tag="acc")

            def mm(tile_, pi):
                kh, kw = divmod(pairs[pi][1], 3)
                rhs4 = tile_[:].rearrange("p (b h w) -> p b h w", b=B, h=6)
                rhs = rhs4[:, :, kh:kh + 4, kw:kw + 32]
                lt = lhsT_t[:, pi * COUT:(pi + 1) * COUT]
                nc.tensor.matmul(
                    acc[:], lt, rhs,
                    start=(pi == 0), stop=(pi == NP - 1))

            ug = []
            for t in range(NGRID_TILES):
                u = upool.tile([128, SLAB_FREE], DT.float32r, tag=f"ug{t}")
                nc.scalar.activation(
                    u[:], x_rep[:], AF.Sigmoid,
                    bias=consts_t[:, 2 * t + 1:2 * t + 2],
                    scale=consts_t[:, 2 * t:2 * t + 1])
                ug.append(u)
            pi = 0
            for t in range(NGRID_TILES):
                for _ in range(9):
                    mm(ug[t], pi)
                    pi += 1

            # risky rows: row = m*(2*u1 - m + 40); engine class round-robin
            for t in range(nm_tiles):
                cls = ("a", "d", "g")[t % 3]
                bias_ap = consts_t[:, 2 * NGRID_TILES + t:
                                   2 * NGRID_TILES + t + 1]
                if cls == "a":
                    u1 = spool.tile([128, SLAB_FREE], DT.float32, tag="u1")
                    nc.scalar.activation(u1[:], x_rep[:], AF.Relu,
                                         bias=bias_ap, scale=1.0 / C)
                    m = spool.tile([128, SLAB_FREE], DT.float32, tag="m")
                    nc.vector.tensor_scalar_min(m[:], u1[:], DELTA)
                    s = spool.tile([128, SLAB_FREE], DT.float32, tag="s")
                    nc.gpsimd.tensor_scalar(s[:], u1[:], 2.0, 40.0,
                                            ALU.mult, ALU.add)
                    s2 = spool.tile([128, SLAB_FREE], DT.float32, tag="s2")
                    nc.vector.tensor_tensor(s2[:], s[:], m[:], ALU.subtract)
                    row = mrpool.tile([128, SLAB_FREE], DT.float32r, tag="mr")
                    nc.vector.tensor_tensor(row[:], m[:], s2[:], ALU.mult)
                else:
                    eng = nc.vector if cls == "d" else nc.gpsimd
                    z = spool.tile([128, SLAB_FREE], DT.float32, tag="z")
                    eng.tensor_scalar(z[:], x_rep[:], 1.0 / C, bias_ap,
                                      ALU.mult, ALU.add)
                    u1 = spool.tile([128, SLAB_FREE], DT.float32, tag="u1")
                    eng.tensor_scalar_max(u1[:], z[:], 0.0)
                    m = spool.tile([128, SLAB_FREE], DT.float32, tag="m")
                    eng.tensor_scalar(m[:], z[:], 0.0, DELTA,
                                      ALU.max, ALU.min)
                    s = spool.tile([128, SLAB_FREE], DT.float32, tag="s")
                    eng.tensor_scalar(s[:], u1[:], 2.0, 40.0,
                                      ALU.mult, ALU.add)
                    s2 = spool.tile([128, SLAB_FREE], DT.float32, tag="s2")
                    eng.tensor_tensor(s2[:], s[:], m[:], ALU.subtract)
                    row = mrpool.tile([128, SLAB_FREE], DT.float32r, tag="mr")
                    eng.tensor_tensor(row[:], m[:], s2[:], ALU.mult)
                while pi < NP and pairs[pi][0] == NGRID_TILES + t:
                    mm(row, pi)
                    pi += 1
            assert pi == NP

            # stats + collective (ACT Identity/Square stay in sigmoid table)
            scr = bpool.tile([COUT, NPIX], DT.float32, tag="scr")
            s1 = bpool.tile([COUT, 1], DT.float32, tag="s1")
            nc.scalar.activation(scr[:], acc[:], AF.Identity, accum_out=s1[:])
            scr2 = bpool.tile([COUT, NPIX], DT.float32, tag="scr2")
            s2t = bpool.tile([COUT, 1], DT.float32, tag="s2t")
            nc.scalar.activation(scr2[:], acc[:], AF.Square, accum_out=s2t[:])
            stats = bpool.tile([COUT, 2], DT.float32, tag="stats")
            nc.vector.tensor_copy(stats[:, 0:1], s1[:])
            nc.vector.tensor_copy(stats[:, 1:2], s2t[:])

            st_in = dram.tile([COUT, 2], DT.float32, tag="sti")
            st_out = dram.tile([COUT, 2], DT.float32, tag="sto")
            nc.sync.dma_start(st_in[:], stats[:])
            if no_cc:
                nc.sync.dma_start(st_out[:], st_in[:])
            else:
                nc.gpsimd.collective_compute(
                    "AllReduce", ALU.add,
                    replica_groups=[list(range(NCORES))],
                    ins=[st_in.opt()], outs=[st_out.opt()])
            gstats = bpool.tile([COUT, 2], DT.float32, tag="gstats")
            nc.sync.dma_start(gstats[:], st_out[:])
            return acc, gstats

        def bn_tail(rep, acc, gstats):
            """BN scalars + normalize, all on DVE (no ACT after collective)."""
            npix_inv = 1.0 / (B * OH * OW)
            mean = bpool.tile([COUT, 1], DT.float32, tag="mean")
            nc.vector.tensor_scalar_mul(mean[:], gstats[:, 0:1], npix_inv)
            msq = bpool.tile([COUT, 1], DT.float32, tag="msq")
            nc.vector.tensor_tensor(msq[:], mean[:], mean[:], ALU.mult)
            y = bpool.tile([COUT, 1], DT.float32, tag="y")
            # y = var + eps = s2/n - mean^2 + eps
            ev2 = bpool.tile([COUT, 1], DT.float32, tag="ev2")
            nc.vector.tensor_scalar(ev2[:], gstats[:, 1:2], npix_inv, BN_EPS,
                                    ALU.mult, ALU.add)
            nc.vector.tensor_tensor(y[:], ev2[:], msq[:], ALU.subtract)
            # rstd = rsqrt(y): bit-hack + 3 Newton iterations
            yi = bpool.tile([COUT, 1], DT.int32, tag="yi")
            nc.vector.tensor_scalar(yi[:], y[:].bitcast(DT.int32), 1, None,
                                    ALU.arith_shift_right)
            r0 = bpool.tile([COUT, 1], DT.int32, tag="r0")
            nc.vector.tensor_tensor(r0[:], gb_t[:, 2:3].bitcast(DT.int32),
                                    yi[:], ALU.subtract)
            yh = bpool.tile([COUT, 1], DT.float32, tag="yh")
            nc.vector.tensor_scalar_mul(yh[:], y[:], 0.5)
            r = r0[:].bitcast(DT.float32)
            for it in range(3):
                rr = bpool.tile([COUT, 1], DT.float32, tag=f"rr{it}")
                nc.vector.tensor_tensor(rr[:], r, r, ALU.mult)
                t2 = bpool.tile([COUT, 1], DT.float32, tag=f"t2{it}")
                nc.vector.tensor_tensor(t2[:], rr[:], yh[:], ALU.mult)
                t3 = bpool.tile([COUT, 1], DT.float32, tag=f"t3{it}")
                nc.vector.tensor_tensor(t3[:], gb_t[:, 3:4], t2[:],
                                        ALU.subtract)
                rn = bpool.tile([COUT, 1], DT.float32, tag=f"rn{it}")
                nc.vector.tensor_tensor(rn[:], r, t3[:], ALU.mult)
                r = rn[:]
            scale_t = bpool.tile([COUT, 1], DT.float32, tag="scale_t")
            nc.vector.tensor_tensor(scale_t[:], r, gb_t[:, 0:1], ALU.mult)
            tmp3 = bpool.tile([COUT, 1], DT.float32, tag="tmp3")
            nc.vector.tensor_tensor(tmp3[:], mean[:], scale_t[:], ALU.mult)
            shift_t = bpool.tile([COUT, 1], DT.float32, tag="shift_t")
            nc.vector.tensor_tensor(shift_t[:], gb_t[:, 1:2], tmp3[:],
                                    ALU.subtract)
            outn = bpool.tile([COUT, NPIX], DT.float32, tag="outn")
            nc.vector.tensor_scalar(outn[:], acc[:], scale_t[:], shift_t[:],
                                    ALU.mult, ALU.add)
            outc = bpool.tile([COUT, NPIX], DT.float32, tag="outc")
            nc.vector.tensor_scalar(outc[:], outn[:], 0.0, 10.0,
                                    ALU.max, ALU.min)
            nc.sync.dma_start(out_d[rep], outc[:])

        prev = None
        for rep in range(reps):
            state = build_rep(rep)
            if prev is not None:
                bn_tail(rep - 1, *prev)
            prev = state
        bn_tail(reps - 1, *prev)

    nc.compile()
    return nc


_CACHE = {}


def _get_program(nm_tiles, pairs, nconst, reps=1, no_cc=False):
    key = (nm_tiles, pairs, nconst, reps, no_cc)
    if key not in _CACHE:
        _CACHE[key] = _build_program(nm_tiles, pairs, nconst, reps=reps,
                                     no_cc=no_cc)
    return _CACHE[key]


def run(x, theta, gamma, beta, reps=1, trace=False):
    prep = _host_prep(x, theta)
    magic = np.full(COUT, np.uint32(0x5F3759DF)).view(np.float32)
    gb = np.stack([np.asarray(gamma, np.float32),
                   np.asarray(beta, np.float32),
                   magic,
                   np.full(COUT, 1.5, np.float32)], axis=0)
    nc = _get_program(prep["nm_tiles"], prep["pairs"], prep["nconst"],
                      reps=reps)
    in_maps = [{
        "xslab": prep["slabs"][s],
        "lhsT": prep["lhsT"],
        "consts": prep["consts"],
        "gb": gb,
    } for s in range(NCORES)]
    res = run_bass_kernel_spmd(nc, in_maps, core_ids=list(range(NCORES)),
                               trace=trace)
    full = np.zeros((B, COUT, OH, OW), np.float32)
    for s in range(NCORES):
        shard = res.results[s]["out"][-1]
        sh = shard.reshape(COUT, B, 4, OW).transpose(1, 0, 2, 3)
        full[:, :, 4 * s:4 * s + 4, :] = sh
    return full, res


def kernel(x, theta, gamma, beta):
    full, _ = run(x, theta, gamma, beta, reps=1)
    return full

